# revision 31
# baseline (speedup 1.0000x reference)
"""BackpropWiSARD v4: f-sharded gather kernel, hash pipelined under gathers.

Strategy (f-parallel, batch replicated), evolved from v2:
  - Core k owns f in [k*28, (k+1)*28): 7 windows of 4 f's (window rows
    fl*8192+e, 32768 rows -> int16 idx). Table stored as sign(+-1) bf16,
    (F,E,C128) row-major per core shard; min-over-h then direct sum gives
    2S-F without an is_ge pass; host sums per-core partials + bias.
  - x_b is permuted by input_order on host (pure input relayout), so the
    device hash reads its bits with one sequential DMA.
  - Index math per window (num_idxs=8192), b = w2*128 + v*16 + u,
    w2 = a*2 + bb:  j = q*128 + r, q = h*16 + v*2 + a, r = fl*32 + bb*16 + u.
    idx tile col j//16 = h*128 + v*16 + a*8 + fl*2 + bb, partition j%16 = u.
    Mc partition j%128 = r (h-free), slots (h, v, a).
  - v4 changes vs v2:
    * whole hash+shuffle is int16 (2x DVE rate): mult products <= 8191 and
      idx = fl*E + e <= 32767 both fit; xq/hvx/rofsx inputs are int16.
    * windows are processed in pipeline GROUPS [1, 2, 2, 2]: each group
      hashes, folds, permutes, replicates only its windows, then issues its
      gathers -- so gathers start ~20us in while the DVE hashes the next
      group. The gather phase (~130us) is HBM random-read bound: 57344
      random 256B reads at ~34ns/access/engine over 16 engines (~120GB/s);
      descriptor GEN is nearly free (calls dispatch in ~85ns, then stall
      on the 4 SWDGE queue rings, ~65 entries deep each).
    * Q7 gather-ucode warmup call (first call pays ~7us IRAM load) off the
      critical path; group-0 inputs DMA'd first; fold/replica DMAs split
      across sync+scalar HWDGE; bf16 output (counts <= 224 exact).
  - Gathers: 8 calls/window of 1024 idxs (ucode cap; 1536/1920/2048 all
    crash the ucode), queues round-robin; DVE min/sum per window pipelined
    via mpool; host folds fl-partitions + bias.
"""

import sys

sys.path.insert(0, "/opt/trn_rl_repo")

import numpy as np
import ml_dtypes

B, C, F, E, H, I = 512, 100, 224, 8192, 4, 28
NCORES = 8
FC = F // NCORES  # 28 f's per core
FL = 4  # f's per window
NW = FC // FL  # 7 windows
GW = FL * E  # 32768 rows per window
CP = 128  # padded classes (256B gather rows)
IP = 32  # XOR-tree width (i=28 data + rofs at 28 + zeros)
W2 = 4  # b//128
NV = 8  # v = (b//16)%8
NU = 16  # u = b%16

NIC = 1024  # idx per gather call (>1024 crashes the gather ucode on this HW)
GROUPS = [[0], [1, 2], [3, 4], [5, 6]]  # window pipeline groups

_NC = None


def _build(loop_reps=1):
    import contextlib

    import concourse.mybir as mybir
    import concourse.tile as tile
    from concourse import bacc
    from concourse.library_config import mlp

    dt = mybir.dt
    op = mybir.AluOpType

    nc = bacc.Bacc(
        "TRN2", target_bir_lowering=False, debug=False, num_swdge_queues=4
    )

    tbl = nc.dram_tensor("tbl", (NW * GW, CP), dt.bfloat16, kind="ExternalInput")
    xq = nc.dram_tensor("xq", (128, NW * W2 * FL * I), dt.int16, kind="ExternalInput")
    hvx = nc.dram_tensor("hvx", (128, H * IP), dt.int16, kind="ExternalInput")
    rofsx = nc.dram_tensor("rofsx", (128, H * W2 * FL), dt.int16, kind="ExternalInput")
    outd = nc.dram_tensor("out", (128, NV * 2 * CP), dt.bfloat16, kind="ExternalOutput")

    with tile.TileContext(nc) as tc:
        nc.gpsimd.load_library(mlp)
        with (
            tc.tile_pool(name="main", bufs=1) as pool,
            tc.tile_pool(name="mc", bufs=4) as mpool,
            (tc.For_i(0, loop_reps, 1) if loop_reps > 1 else contextlib.nullcontext()),
        ):
            # group-0 hash inputs load first (xq-g0 on sync, hvx+rofs on
            # scalar, in parallel) so hash g0 starts ~9us in; later xq
            # groups follow
            xq_sb = pool.tile([128, NW, W2, FL, I], dt.int16)
            WCOLS = W2 * FL * I
            hvx_sb = pool.tile([128, H, 1, 1, IP], dt.int16)
            rofs_sb = pool.tile([128, H * W2 * FL], dt.int16)
            w0, w1 = GROUPS[0][0], GROUPS[0][-1] + 1
            nc.sync.dma_start(
                out=xq_sb[:, w0:w1].rearrange("p w t fl i -> p (w t fl i)"),
                in_=xq.ap()[:, w0 * WCOLS : w1 * WCOLS],
            )
            nc.scalar.dma_start(
                out=hvx_sb[:].rearrange("p h o z i -> p (h o z i)"), in_=hvx.ap()
            )
            nc.scalar.dma_start(out=rofs_sb[:], in_=rofsx.ap())
            for ws in GROUPS[1:]:
                w0, w1 = ws[0], ws[-1] + 1
                nc.sync.dma_start(
                    out=xq_sb[:, w0:w1].rearrange("p w t fl i -> p (w t fl i)"),
                    in_=xq.ap()[:, w0 * WCOLS : w1 * WCOLS],
                )

            # warm the Q7 gather ucode (first call pays ~6-8us IRAM load;
            # the first call on each queue pays queue setup) with tiny
            # num_idxs=16 gathers of row 0, off the critical path
            wdix = pool.tile([128, 1], dt.int16)
            nc.gpsimd.memset(wdix[:], 0)
            wscr = pool.tile([128, 1, CP], dt.bfloat16)
            nc.gpsimd.dma_gather(
                out_ap=wscr[:],
                in_ap=tbl.ap()[0:GW, :],
                idxs_ap=wdix[:],
                num_idxs=16,
                num_idxs_reg=16,
                elem_size=CP,
                # queue 3: Tile's DMASW lane rotation puts later lane-0
                # users (calls j % 4 == 3) on queue 3; a sem lane must
                # always be updated from the same SWDGE queue
                queue_num=3,
            )

            idx16 = pool.tile([128, NW, H, 2, FL, 2], dt.int16)  # (w,h,a,fl,bb)
            fold16 = pool.tile([16, NV, NW, H, 2, FL, 2], dt.int16)
            pT = pool.tile([16, NW, H, NV, 2, FL, 2], dt.int16)
            idxT = pool.tile([128, NW, H, NV, 2, FL, 2], dt.int16)

            acc = pool.tile([128, NV, 2, CP], dt.bfloat16)

            ncall = 0

            def hash_group(gi, ws):
                # per-group msk tile: group g+1's hash must not WAR-stall on
                # group g's tree still being read by the emission copy
                msk = pool.tile([128, H, W2, FL, IP], dt.int16, tag=f"msk{gi}")
                nc.vector.memset(msk[:, :, :, :, I + 1 : IP], 0)
                for w in ws:
                    nc.vector.tensor_copy(
                        out=msk[:, :, :, :, I : I + 1].rearrange(
                            "p h t fl z -> p (h t fl z)"
                        ),
                        in_=rofs_sb[:],
                    )
                    for h in range(H):
                        hv_h = hvx_sb[:, h, :, :, 0:I].to_broadcast([128, W2, FL, I])
                        nc.vector.tensor_tensor(
                            out=msk[:, h, :, :, 0:I],
                            in0=xq_sb[:, w, :, :, :],
                            in1=hv_h,
                            op=op.mult,
                        )
                    width = IP
                    while width > 1:
                        width //= 2
                        nc.vector.tensor_tensor(
                            out=msk[:, :, :, :, 0:width],
                            in0=msk[:, :, :, :, 0:width],
                            in1=msk[:, :, :, :, width : 2 * width],
                            op=op.bitwise_xor,
                        )
                    nc.vector.tensor_copy(
                        out=idx16[:, w, :, :, :, :],
                        in_=msk[:, :, :, :, 0].rearrange(
                            "p h (a bb) fl -> p h a fl bb", a=2
                        ),
                    )
                w0, w1 = ws[0], ws[-1] + 1
                # fold + replicate DMAs split across sync and scalar HWDGE
                # engines to halve the serial dispatch latency
                for v in range(NV):
                    eng = nc.sync if v % 2 == 0 else nc.scalar
                    eng.dma_start(
                        out=fold16[:, v, w0:w1, :, :, :, :].rearrange(
                            "u w h a fl bb -> u (w h a fl bb)"
                        ),
                        in_=idx16[v * 16 : (v + 1) * 16, w0:w1, :, :, :, :].rearrange(
                            "u w h a fl bb -> u (w h a fl bb)"
                        ),
                    )
                nc.vector.tensor_copy(
                    out=pT[:, w0:w1],
                    in_=fold16[:, :, w0:w1].rearrange(
                        "u v w h a fl bb -> u w h v a fl bb"
                    ),
                )
                for r in range(8):
                    eng = nc.sync if r % 2 == 0 else nc.scalar
                    eng.dma_start(
                        out=idxT[r * 16 : (r + 1) * 16, w0:w1].rearrange(
                            "u w h v a fl bb -> u (w h v a fl bb)"
                        ),
                        in_=pT[:, w0:w1].rearrange(
                            "u w h v a fl bb -> u (w h v a fl bb)"
                        ),
                    )

            # split a window's 8192 idxs into NIC-sized calls (+ remainder);
            # NIC=1920 keeps descs/ring at 120+1 sem <= 128 ring entries
            splits = []
            s = 0
            while s < 8192:
                n = min(NIC, 8192 - s)
                splits.append((s, n))
                s += n

            def gather_group(ws):
                """Enqueue the gather calls for ws; return the Mc tiles."""
                nonlocal ncall
                mcs = []
                for w in ws:
                    Mc = mpool.tile([128, H, NV, 2, CP], dt.bfloat16, tag="Mc")
                    mcs.append(Mc)
                    mcf = Mc[:].rearrange("p h v a c -> p (h v a) c")
                    idxf = idxT[:, w].rearrange("p h v a fl bb -> p (h v a fl bb)")
                    for s0, n in splits:
                        nc.gpsimd.dma_gather(
                            out_ap=mcf[:, s0 // 128 : (s0 + n) // 128, :],
                            in_ap=tbl.ap()[w * GW : (w + 1) * GW, :],
                            idxs_ap=idxf[:, s0 // 16 : (s0 + n) // 16],
                            num_idxs=n,
                            num_idxs_reg=n,
                            elem_size=CP,
                            queue_num=ncall % 4,
                        )
                        ncall += 1
                return mcs

            def minsum_group(mcs):
                for Mc in mcs:
                    nc.vector.tensor_tensor(
                        out=Mc[:, 0:2], in0=Mc[:, 0:2], in1=Mc[:, 2:4], op=op.min
                    )
                    nc.vector.tensor_tensor(
                        out=Mc[:, 0:1], in0=Mc[:, 0:1], in1=Mc[:, 1:2], op=op.min
                    )
                    nc.vector.tensor_tensor(
                        out=acc[:], in0=acc[:], in1=Mc[:, 0, :, :, :], op=op.add
                    )

            # software pipeline: DVE order is [hash g, minsum g-1] so the
            # hash of the next group never queues behind a DMA-stalled
            # minsum; pool gathers run continuously.
            hash_group(0, GROUPS[0])
            nc.vector.memset(acc[:], 0)  # after hash g0: off the critical path
            prev_mcs = gather_group(GROUPS[0])
            for gi in range(1, len(GROUPS)):
                hash_group(gi, GROUPS[gi])
                minsum_group(prev_mcs)
                prev_mcs = gather_group(GROUPS[gi])
            minsum_group(prev_mcs)

            nc.sync.dma_start(
                out=outd.ap(), in_=acc[:].rearrange("p v a c -> p (v a c)")
            )

    nc.compile()
    return nc


def get_nc(loop_reps=1):
    global _NC
    if loop_reps != 1:
        return _build(loop_reps)
    if _NC is None:
        _NC = _build()
    return _NC


def prep_in_maps(inputs):
    x_b = np.asarray(inputs["x_b"], dtype=np.int32)
    input_order = np.asarray(inputs["input_order"], dtype=np.int32)
    hash_values = np.asarray(inputs["hash_values"], dtype=np.int32)
    table = np.asarray(inputs["table"], dtype=np.float32)

    # sign table (C,F,E) -> (F,E,CP) +-1 bf16, zero-pad classes
    tp = np.zeros((F, E, CP), dtype=ml_dtypes.bfloat16)
    tp[:, :, :C] = np.where(table >= 0, np.float32(1.0), np.float32(-1.0)).transpose(
        1, 2, 0
    )

    # mapped bits, host-permuted: (B, F, I)
    xm = x_b[:, input_order].reshape(B, F, I).astype(np.int16)

    hvp = np.zeros((H, IP), dtype=np.int16)
    hvp[:, :I] = hash_values
    hvx = np.ascontiguousarray(np.tile(hvp.reshape(1, H * IP), (128, 1)))

    # rofs slab value fl*E for msk[:, h, w2, fl, 28]
    rofs = np.tile(
        (np.arange(FL, dtype=np.int16) * E).reshape(1, 1, 1, FL), (128, H, W2, 1)
    ).reshape(128, H * W2 * FL)
    rofs = np.ascontiguousarray(rofs)

    in_maps = []
    for k in range(NCORES):
        tt = np.ascontiguousarray(tp[k * FC : (k + 1) * FC].reshape(NW * GW, CP))
        # xq[p, w, w2, fl, i] = xm[w2*128+p, k*28+w*4+fl, i]
        xk = (
            xm[:, k * FC : (k + 1) * FC, :]
            .reshape(W2, 128, NW, FL, I)
            .transpose(1, 2, 0, 3, 4)
        )
        xk = np.ascontiguousarray(xk).reshape(128, NW * W2 * FL * I)
        in_maps.append({"tbl": tt, "xq": xk, "hvx": hvx, "rofsx": rofs})
    return in_maps


def finish_output(parts, bias):
    """parts[k]: (128, NV*2*CP) f32 acc for core k; fold fl partitions, sum
    cores, reorder b, add bias."""
    total = np.zeros((2, 2, NV, NU, CP), dtype=np.float32)  # (a, bb, v, u, c)
    for k in range(NCORES):
        acc = np.asarray(parts[k], dtype=np.float32).reshape(
            FL, 2, NU, NV, 2, CP
        )  # (fl, bb, u, v, a, c)
        total += acc.sum(axis=0).transpose(3, 0, 2, 1, 4)
    # b = w2*128 + v*16 + u, w2 = a*2 + bb
    counts = total.reshape(B, CP)[:, :C]
    return counts + np.asarray(bias, dtype=np.float32).reshape(1, C)


def kernel(**inputs):
    from concourse.bass_utils import run_bass_kernel_spmd

    nc = get_nc()
    in_maps = prep_in_maps(inputs)
    res = run_bass_kernel_spmd(nc, in_maps, list(range(NCORES)))
    parts = [res.results[k]["out"] for k in range(NCORES)]
    return finish_output(parts, inputs["bias"]).astype(np.float32)
